# revision 34
# baseline (speedup 1.0000x reference)
"""DynamicSparseMoE grouped-GEMM kernel for 8 TRN2 NeuronCores — raw bass.

out[t] = tokens[t] @ weight[exp_ids[t]]   (T=8192, E=8, D=2048 -> 2048)

Strategy (expert-parallel, host-side dispatch):
  - Host sorts tokens by expert; core e owns expert e's weight and its
    routed tokens, padded to a common capacity C (SPMD needs equal shapes).
  - fp16 compute (PE 1 cyc/row), fp32 PSUM accumulation.
  - Tokens are the stationary operand (xT blocks [128 d, 128 t]); the
    weight is the moving operand in 512-wide o-slices.
  - DRAM input layouts mirror SBUF exactly (host pre-packs), so every
    DMA is one contiguous chunked copy on a HWDGE ring.
  - Startup: the early HBM stream ramps slowly (~150-300 GB/s for the
    first ~6 us), so blocks 0-3 run as a quad with os-slice-major phases
    (one 512-wide o-slice across 4 stationary blocks per phase, kb
    outermost): weight demand is 128 KB per 853 ns sweep (~150 GB/s),
    which the ramp can feed; the weight DRAM layout is os-major so the
    stream is consumed strictly in order.  ~44 warmup matmuls on a
    memset tile bridge the fixed ~7 us program prologue until the first
    weight chunk lands.
  - The packed final block (<=64 real tokens, two column-group-packed
    concurrent matmul pairs via PSUM base partition 0/64) runs right
    after the quad so the kernel does not end on it; remaining blocks
    run as per-(block, o-slice) units with kb innermost: each unit
    accumulates one PSUM bank, which is copied out and streamed to DRAM
    while later units compute (continuous output drain; the last block
    streams per-o-slice so the final transfer is only 128 KB).
  - DMA completion semaphores rotate through 8 sems per HWDGE ring
    (sync carries x + final-block stores, scalar carries weights +
    per-block output stores). One shared sem per stream is UNSOUND: a
    DMA's +16 comes from 16 independent SDMA engines, so a fast engine's
    increments from chunk k+1 can reach a threshold while a slow engine
    is still writing chunk k (observed as intermittent corruption of the
    first accumulation). The 8-sem rotation gives every wait one
    tracked transfer.
  - All semaphores are cleared at program end (the NEFF is executed more
    than once per session; sems must return to 0).
"""

import os
from contextlib import ExitStack

import numpy as np

# A previously wedged NeuronCore (NRT_EXEC_UNIT_UNRECOVERABLE) recovers on
# the next init when core reset is requested; must be set before NRT init.
os.environ.setdefault("NEURON_RT_RESET_CORES", "1")

P = 128
D = 2048
E = 8
KB = D // P  # 16 contraction blocks
NS = 512  # o-slice width (one PSUM bank)
NOS = 4
# Warmups bridge the ~6.7 us fixed prologue to the first weight arrival
# (10.5-13 us depending on per-core HBM contention luck). Too few lets
# the HAM clock-gate re-throttle on slow-start cores (first ~3.4 us of
# real matmuls at half clock); extra ones only delay fast cores ~0.5 us.
N_WARM = 84

_cache = {}


def _ensure_imports():
    try:
        import concourse.bass  # noqa: F401
    except ImportError:
        import sys

        for p in ("/opt/trn_rl_repo", "/opt/pypackages"):
            if p not in sys.path:
                sys.path.append(p)


def _build(C, last_m=128):
    """Build + compile the per-core raw-bass program for capacity C."""
    _ensure_imports()
    import concourse.bacc as bacc
    import concourse.mybir as mybir

    f16 = mybir.dt.float16
    f32 = mybir.dt.float32
    TB = C // P
    assert TB >= 5, C
    packed = last_m == 64
    FB = TB - 1 if packed else None  # packed final block index
    blocks = [b for b in range(4, TB) if b != FB]  # regular per-block units
    QX = KB * 512  # xs columns of the quad (blocks 0-3)

    nc = bacc.Bacc(None, target_bir_lowering=False, debug=False)

    xs_d = nc.declare_dram_parameter("xs", [P, KB * C], f16, isOutput=False)
    w_d = nc.declare_dram_parameter("w", [P, NOS * KB * NS], f16, isOutput=False)
    out_d = nc.declare_dram_parameter("out", [C, D], f16, isOutput=True)

    # xs column offset of stationary block (b, kb); the quad (blocks 0-3)
    # is kb-major with 512 token-columns per kb, blocks 4+ block-major in
    # DMA order (packed final block first).
    border = ([FB] if packed else []) + blocks

    def xoff(b, kb):
        if b < 4:
            return kb * 512 + b * P
        return QX + border.index(b) * KB * P + kb * P

    def w_sl(t, osl, kb):
        off = (osl * KB + kb) * NS
        return t[:, off : off + NS]

    es = ExitStack()
    with es:
        xs = es.enter_context(nc.sbuf_tensor("xs_sb", [P, KB * C], f16))
        wt = es.enter_context(nc.sbuf_tensor("w_sb", [P, NOS * KB * NS], f16))
        o_sb = es.enter_context(nc.sbuf_tensor("o_sb", [P, 6 * D], f16))
        warm = es.enter_context(nc.sbuf_tensor("warm", [P, 64], f16))
        ps = [
            es.enter_context(nc.psum_tensor(f"ps{i}", [P, NS], f32)) for i in range(8)
        ]
        sems = {}
        for name in (
            "ws",  # vector memset -> tensor warmups
            "mm",  # tensor unit done -> vector
            "cp",  # vector unit copies done -> tensor bank reuse
            "co",  # vector output chunk assembled -> scalar store
            "cf",  # vector final-block copies -> sync store
        ):
            sems[name] = es.enter_context(nc.semaphore(f"s_{name}"))

        # DMA completion tracking: a DMA's sem gets +16 from 16 INDEPENDENT
        # SDMA engines, so with several DMAs on one sem a fast engine's
        # increments from chunk k+1 can reach the threshold while a slow
        # engine is still writing chunk k. Rotate 8 sems per ring (reuse
        # distance 8 transfers) so each wait tracks one specific DMA —
        # the same scheme Tile's DMAHW lanes use.
        class Ring:
            def __init__(self, eng, tag):
                self.eng = eng
                self.pool = [
                    es.enter_context(nc.semaphore(f"{tag}{i}")) for i in range(8)
                ]
                self.n = 0
                self.waits = []  # per-DMA (sem, value)

            def dma(self, dst, src):
                s = self.pool[self.n % 8]
                v = 16 * (self.n // 8 + 1)
                self.eng.dma_start(dst, src).then_inc(s, 16)
                self.waits.append((s, v))
                self.n += 1
                return (s, v)

            def wait_all(self):
                final = {}
                for s, v in self.waits:
                    final[s.num] = (s, v)
                for s, v in final.values():
                    self.eng.wait_ge(s, v)

        # ---- unit table ----
        # ("qp", phase_os, banks) | ("pf", b, (os,os), banks) | ("bl", b, os, bank)
        units = []
        for p in range(NOS):
            units.append(("qp", None, p, [(p % 2) * 4 + i for i in range(4)]))
        if packed:
            units.append(("pf", FB, (0, 1), [0, 1]))
            units.append(("pf", FB, (2, 3), [2, 3]))
        bank_cursor = 4 if packed else 0
        for b in blocks:
            for osl in range(NOS):
                units.append(("bl", b, osl, [bank_cursor % 8]))
                bank_cursor += 1
        last_block = blocks[-1]

        # o_sb buffer per block: quad -> 0-3, packed final -> 5,
        # regular blocks cycle {0,1,2,4} (reuse gated on osc).
        def obuf(b):
            if b < 4:
                return b
            if b == FB:
                return 5
            return [0, 1, 2, 4][blocks.index(b) % 4]

        # ---------------- DMA streams: sync = x, scalar = weights+outs -
        # (a third SWDGE queue on gpsimd was tried for the startup and
        # measured ~7 us SLOWER: Q7 descriptor generation serializes and
        # shares the same SDMA engines.)
        # Both rings share the ~360 GB/s per-core HBM budget for the whole
        # stream (~180 each measured), so: all weight phases are chunked
        # at 2-kb granularity (256 KB) with per-2kb waits — a coarse 1 MB
        # os1 chunk behind os0 on the scalar ring was measured arriving
        # ~2 us after qp1 needed it — and the os3 phase's weights ride
        # the otherwise-lighter sync ring (sync 6.75 MB vs scalar 10.4).
        sy_ring = Ring(nc.sync, "sy")
        sc_ring = Ring(nc.scalar, "sc")
        # Leading chunks are 1 kb (128 KB) so the first matmul's two
        # gating transfers land ~0.8 us sooner at the slow early rate;
        # the rest are 2-kb chunks. xq_w/w-os0 waits are indexed per kb
        # for kb<2, per 2-kb beyond.
        xq_w = [None] * 9
        w_w = [None] * 33  # w_w[0..8] = os0 (kb0, kb1, then 2-kb); 8/os after
        for kb0, kb1 in [(0, 1), (1, 2)] + [(2 * c, 2 * c + 2) for c in range(1, 8)]:
            xsl = slice(kb0 * 512, kb1 * 512)
            xq_w[kb0 if kb0 < 2 else kb0 // 2 + 1] = sy_ring.dma(
                xs[:, xsl], xs_d[:, xsl]
            )
        for c in range(8):  # os3 weight chunks on the sync ring
            wsl = slice((3 * KB + 2 * c) * NS, (3 * KB + 2 * (c + 1)) * NS)
            w_w[25 + c] = sy_ring.dma(wt[:, wsl], w_d[:, wsl])
        xb_w = {}
        for i, b in enumerate(border):
            xsl = slice(QX + i * KB * P, QX + (i + 1) * KB * P)
            xb_w[b] = sy_ring.dma(xs[:, xsl], xs_d[:, xsl])
        if packed:
            fbuf = obuf(FB) * D
            for osl in range(NOS):
                rows = slice(0, 64) if osl % 2 == 0 else slice(64, P)
                nc.sync.wait_ge(sems["cf"], 1 + osl // 2)
                sy_ring.dma(
                    out_d[FB * P : FB * P + 64, osl * NS : (osl + 1) * NS],
                    o_sb[rows, fbuf + osl * NS : fbuf + (osl + 1) * NS],
                )
        sy_ring.wait_all()

        for kb0, kb1 in [(0, 1), (1, 2)] + [(2 * c, 2 * c + 2) for c in range(1, 8)]:
            wsl = slice(kb0 * NS, kb1 * NS)  # os0, leading 1-kb chunks
            w_w[kb0 if kb0 < 2 else kb0 // 2 + 1] = sc_ring.dma(
                wt[:, wsl], w_d[:, wsl]
            )
        for osl in range(1, 3):  # os1-os2 weight chunks on the scalar ring
            for c in range(8):
                wsl = slice(
                    (osl * KB + 2 * c) * NS, (osl * KB + 2 * (c + 1)) * NS
                )
                w_w[1 + osl * 8 + c] = sc_ring.dma(wt[:, wsl], w_d[:, wsl])
        co_thr = 0
        out_w = {}  # block -> wait index of its (last) out DMA
        for b in [0, 1, 2, 3] + blocks:
            buf = obuf(b) * D
            if b == last_block:
                for osl in range(NOS):
                    co_thr += 1
                    nc.scalar.wait_ge(sems["co"], co_thr)
                    out_w[b] = sc_ring.dma(
                        out_d[b * P : (b + 1) * P, osl * NS : (osl + 1) * NS],
                        o_sb[:, buf + osl * NS : buf + (osl + 1) * NS],
                    )
            else:
                co_thr += 1
                nc.scalar.wait_ge(sems["co"], co_thr)
                out_w[b] = sc_ring.dma(
                    out_d[b * P : (b + 1) * P, :], o_sb[:, buf : buf + D]
                )
        sc_ring.wait_all()

        # ---------------- vector engine: PSUM->SBUF copies -------------
        ve = nc.vector
        ve.memset(warm[:], 0.0).then_inc(sems["ws"], 1)
        for u, (kind, b, osl, banks) in enumerate(units):
            ve.wait_ge(sems["mm"], u + 1)
            if kind == "qp":
                p = osl
                last = None
                for ti in range(4):
                    last = ve.tensor_copy(
                        o_sb[:, ti * D + p * NS : ti * D + (p + 1) * NS],
                        ps[banks[ti]][:],
                    )
                last.then_inc(sems["cp"], 1)
                if p == NOS - 1:
                    ve.nop().then_inc(sems["co"], 4)
            elif kind == "pf":
                buf = obuf(b) * D  # dedicated buf 5: no reuse wait needed
                last = None
                for j, o in enumerate(osl):
                    rows = slice(0, 64) if o % 2 == 0 else slice(64, P)
                    last = ve.tensor_copy(
                        o_sb[rows, buf + o * NS : buf + (o + 1) * NS],
                        ps[banks[j]][rows, :],
                    )
                last.then_inc(sems["cp"], 1)
                ve.nop().then_inc(sems["cf"], 1)
            else:  # bl
                bi = blocks.index(b)
                if osl == 0 and bi != 3:
                    # o_sb buf reuse: bufs cycle {0,1,2,4}; bi 0..2 reuse
                    # quad blocks 0..2's bufs, bi 3 gets the fresh buf 4,
                    # bi>=4 reuses regular block bi-4's buf. Wait for that
                    # specific block's out-DMA completion.
                    prev = bi if bi < 3 else blocks[bi - 4]
                    ve.wait_ge(*out_w[prev])
                buf = obuf(b) * D
                last = ve.tensor_copy(
                    o_sb[:, buf + osl * NS : buf + (osl + 1) * NS], ps[banks[0]][:]
                )
                last.then_inc(sems["cp"], 1)
                if b == last_block:
                    ve.nop().then_inc(sems["co"], 1)
                elif osl == 3:
                    ve.nop().then_inc(sems["co"], 1)

        # ---------------- tensor engine: warmups + matmul stream -------
        te = nc.tensor
        te.wait_ge(sems["ws"], 1)
        for _ in range(N_WARM):
            te.matmul(
                ps[0][:64, :64], lhsT=warm[:, :64], rhs=warm[:, :64],
                start=True, stop=True,
            )
        last_use = {}
        for u, (kind, b, osl, banks) in enumerate(units):
            need = 0
            for bk in banks:
                if bk in last_use:
                    need = max(need, last_use[bk] + 1)
                last_use[bk] = u
            if need:
                te.wait_ge(sems["cp"], need)
            if kind == "qp":
                p = osl
                for kb in range(KB):
                    ci = kb if kb < 2 else kb // 2 + 1  # os0/xq chunk index
                    if p == 0:
                        if kb < 2 or kb % 2 == 0:
                            te.wait_ge(*w_w[ci])
                            te.wait_ge(*xq_w[ci])
                    elif kb % 2 == 0:
                        te.wait_ge(*w_w[1 + p * 8 + kb // 2])
                    for ti in range(4):
                        mm = te.matmul(
                            ps[banks[ti]][:],
                            lhsT=xs[:, xoff(ti, kb) : xoff(ti, kb) + P],
                            rhs=w_sl(wt, p, kb),
                            start=(kb == 0),
                            stop=(kb == KB - 1),
                        )
                mm.then_inc(sems["mm"], 1)
            elif kind == "pf":
                if osl[0] == 0:
                    te.wait_ge(*xb_w[b])
                for kb in range(KB):
                    for j, o in enumerate(osl):
                        dst = ps[banks[j]][:64, :] if o % 2 == 0 else ps[banks[j]][64:, :]
                        mm = te.matmul(
                            dst,
                            lhsT=xs[:, xoff(b, kb) : xoff(b, kb) + 64],
                            rhs=w_sl(wt, o, kb),
                            start=(kb == 0),
                            stop=(kb == KB - 1),
                        )
                mm.then_inc(sems["mm"], 1)
            else:  # bl
                if osl == 0:
                    te.wait_ge(*xb_w[b])
                for kb in range(KB):
                    mm = te.matmul(
                        ps[banks[0]][:],
                        lhsT=xs[:, xoff(b, kb) : xoff(b, kb) + P],
                        rhs=w_sl(wt, osl, kb),
                        start=(kb == 0),
                        stop=(kb == KB - 1),
                    )
                mm.then_inc(sems["mm"], 1)

        # ---------------- teardown: reset sems for the next execution --
        # The NEFF executes more than once per load; sems must return to
        # zero. (A full gpsimd.dma_reset drain costs ~25 us — measured —
        # so only clear the sem values; all DMAs were completion-waited
        # above.)
        # Per-sem clears: a single range sem_clear was measured ~22 us
        # SLOWER (it lowers to a slow ucode path); 21 singles cost ~1 us.
        nc.all_engine_barrier()
        for s in list(sems.values()) + sy_ring.pool + sc_ring.pool:
            nc.gpsimd.sem_clear(s)

        nc.compile()
    return nc


def _get_nc(C, last_m):
    key = (C, last_m)
    if key not in _cache:
        _cache[key] = _build(C, last_m)
    return _cache[key]


def kernel(tokens, weight, exp_ids, _trace=False):
    _ensure_imports()
    from concourse.bass_utils import run_bass_kernel_spmd

    tokens = np.asarray(tokens)
    weight = np.asarray(weight)
    exp_ids = np.asarray(exp_ids)
    T = tokens.shape[0]

    order = np.argsort(exp_ids, kind="stable")
    counts = np.bincount(exp_ids, minlength=E)
    C = max(int(-(-counts.max() // P) * P), 640)
    TB = C // P

    rest = int(counts.max()) - (TB - 1) * P
    last_m = 64 if (TB >= 6 and rest <= 64) else 128
    FB = TB - 1 if last_m == 64 else None
    border = ([FB] if FB is not None else []) + [
        b for b in range(4, TB) if b != FB
    ]

    starts = np.zeros(E + 1, dtype=np.int64)
    np.cumsum(counts, out=starts[1:])

    tokens_c = tokens.astype(np.float16)
    weight_c = weight.astype(np.float16)

    in_maps = []
    for e in range(E):
        idx = order[starts[e] : starts[e + 1]]
        xt = np.zeros((D, C), dtype=np.float16)
        xt[:, : counts[e]] = tokens_c[idx].T
        xt3 = xt.reshape(KB, P, C)
        parts = [np.ascontiguousarray(xt3[:, :, :512].transpose(1, 0, 2)).reshape(P, -1)]
        for b in border:
            parts.append(
                np.ascontiguousarray(
                    xt3[:, :, b * P : (b + 1) * P].transpose(1, 0, 2)
                ).reshape(P, -1)
            )
        xs = np.concatenate(parts, axis=1)
        # weight os-major: [os 4][kb 16][128 p, 512] packed to [128, 4*16*512]
        w4 = weight_c[e].reshape(KB, P, NOS, NS)
        ww = np.ascontiguousarray(w4.transpose(1, 2, 0, 3)).reshape(P, -1)
        in_maps.append({"xs": xs, "w": ww})

    nc = _get_nc(C, last_m)
    res = run_bass_kernel_spmd(
        nc,
        in_maps,
        core_ids=list(range(E)),
        trace=_trace,
        trace_cores=list(range(E)) if _trace else None,
    )

    out = np.empty((T, D), dtype=np.float32)
    for e in range(E):
        idx = order[starts[e] : starts[e + 1]]
        out[idx] = res.results[e]["out"][: counts[e], :].astype(np.float32)
    if _trace:
        return out, res
    return out


# revision 35
# speedup vs baseline: 1.0120x; 1.0120x over previous
"""DynamicSparseMoE grouped-GEMM kernel for 8 TRN2 NeuronCores — raw bass.

out[t] = tokens[t] @ weight[exp_ids[t]]   (T=8192, E=8, D=2048 -> 2048)

Strategy (expert-parallel, host-side dispatch):
  - Host sorts tokens by expert; core e owns expert e's weight and its
    routed tokens, padded to a common capacity C (SPMD needs equal shapes).
  - fp16 compute (PE 1 cyc/row), fp32 PSUM accumulation.
  - Tokens are the stationary operand (xT blocks [128 d, 128 t]); the
    weight is the moving operand in 512-wide o-slices.
  - DRAM input layouts mirror SBUF exactly (host pre-packs), so every
    DMA is one contiguous chunked copy on a HWDGE ring.
  - Startup: the early HBM stream ramps slowly (~150-300 GB/s for the
    first ~6 us), so blocks 0-3 run as a quad with os-slice-major phases
    (one 512-wide o-slice across 4 stationary blocks per phase, kb
    outermost): weight demand is 128 KB per 853 ns sweep (~150 GB/s),
    which the ramp can feed; the weight DRAM layout is os-major so the
    stream is consumed strictly in order.  ~44 warmup matmuls on a
    memset tile bridge the fixed ~7 us program prologue until the first
    weight chunk lands.
  - The packed final block (<=64 real tokens, two column-group-packed
    concurrent matmul pairs via PSUM base partition 0/64) runs right
    after the quad so the kernel does not end on it; remaining blocks
    run as per-(block, o-slice) units with kb innermost: each unit
    accumulates one PSUM bank, which is copied out and streamed to DRAM
    while later units compute (continuous output drain; the last block
    streams per-o-slice so the final transfer is only 128 KB).
  - DMA completion semaphores rotate through 8 sems per HWDGE ring
    (sync carries x + final-block stores, scalar carries weights +
    per-block output stores). One shared sem per stream is UNSOUND: a
    DMA's +16 comes from 16 independent SDMA engines, so a fast engine's
    increments from chunk k+1 can reach a threshold while a slow engine
    is still writing chunk k (observed as intermittent corruption of the
    first accumulation). The 8-sem rotation gives every wait one
    tracked transfer.
  - All semaphores are cleared at program end (the NEFF is executed more
    than once per session; sems must return to 0).
"""

import os
from contextlib import ExitStack

import numpy as np

# A previously wedged NeuronCore (NRT_EXEC_UNIT_UNRECOVERABLE) recovers on
# the next init when core reset is requested; must be set before NRT init.
os.environ.setdefault("NEURON_RT_RESET_CORES", "1")

P = 128
D = 2048
E = 8
KB = D // P  # 16 contraction blocks
NS = 512  # o-slice width (one PSUM bank)
NOS = 4
# Warmups bridge the ~6.7 us fixed prologue to the first weight arrival
# (10.5-13 us depending on per-core HBM contention luck). Too few lets
# the HAM clock-gate re-throttle on slow-start cores (first ~3.4 us of
# real matmuls at half clock); extra ones only delay fast cores ~0.5 us.
N_WARM = 84

_cache = {}


def _ensure_imports():
    try:
        import concourse.bass  # noqa: F401
    except ImportError:
        import sys

        for p in ("/opt/trn_rl_repo", "/opt/pypackages"):
            if p not in sys.path:
                sys.path.append(p)


def _build(C, last_m=128):
    """Build + compile the per-core raw-bass program for capacity C."""
    _ensure_imports()
    import concourse.bacc as bacc
    import concourse.mybir as mybir

    f16 = mybir.dt.float16
    f32 = mybir.dt.float32
    TB = C // P
    assert TB >= 5, C
    packed = last_m == 64
    FB = TB - 1 if packed else None  # packed final block index
    blocks = [b for b in range(4, TB) if b != FB]  # regular per-block units
    QX = KB * 512  # xs columns of the quad (blocks 0-3)

    nc = bacc.Bacc(None, target_bir_lowering=False, debug=False)

    xs_d = nc.declare_dram_parameter("xs", [P, KB * C], f16, isOutput=False)
    w_d = nc.declare_dram_parameter("w", [P, NOS * KB * NS], f16, isOutput=False)
    out_d = nc.declare_dram_parameter("out", [C, D], f16, isOutput=True)

    # xs column offset of stationary block (b, kb); the quad (blocks 0-3)
    # is kb-major with 512 token-columns per kb, blocks 4+ block-major in
    # DMA order (packed final block first).
    border = ([FB] if packed else []) + blocks

    def xoff(b, kb):
        if b < 4:
            return kb * 512 + b * P
        return QX + border.index(b) * KB * P + kb * P

    def w_sl(t, osl, kb):
        off = (osl * KB + kb) * NS
        return t[:, off : off + NS]

    es = ExitStack()
    with es:
        xs = es.enter_context(nc.sbuf_tensor("xs_sb", [P, KB * C], f16))
        wt = es.enter_context(nc.sbuf_tensor("w_sb", [P, NOS * KB * NS], f16))
        o_sb = es.enter_context(nc.sbuf_tensor("o_sb", [P, 6 * D], f16))
        warm = es.enter_context(nc.sbuf_tensor("warm", [P, 64], f16))
        ps = [
            es.enter_context(nc.psum_tensor(f"ps{i}", [P, NS], f32)) for i in range(8)
        ]
        sems = {}
        for name in (
            "ws",  # vector memset -> tensor warmups
            "mm",  # tensor unit done -> vector
            "cp",  # vector unit copies done -> tensor bank reuse
            "co",  # vector output chunk assembled -> scalar store
            "cf",  # vector final-block copies -> sync store
        ):
            sems[name] = es.enter_context(nc.semaphore(f"s_{name}"))

        # DMA completion tracking: a DMA's sem gets +16 from 16 INDEPENDENT
        # SDMA engines, so with several DMAs on one sem a fast engine's
        # increments from chunk k+1 can reach the threshold while a slow
        # engine is still writing chunk k. Rotate 8 sems per ring (reuse
        # distance 8 transfers) so each wait tracks one specific DMA —
        # the same scheme Tile's DMAHW lanes use.
        class Ring:
            def __init__(self, eng, tag):
                self.eng = eng
                self.pool = [
                    es.enter_context(nc.semaphore(f"{tag}{i}")) for i in range(8)
                ]
                self.n = 0
                self.waits = []  # per-DMA (sem, value)

            def dma(self, dst, src):
                s = self.pool[self.n % 8]
                v = 16 * (self.n // 8 + 1)
                self.eng.dma_start(dst, src).then_inc(s, 16)
                self.waits.append((s, v))
                self.n += 1
                return (s, v)

            def wait_all(self):
                final = {}
                for s, v in self.waits:
                    final[s.num] = (s, v)
                for s, v in final.values():
                    self.eng.wait_ge(s, v)

        # ---- unit table ----
        # ("qp", phase_os, banks) | ("pf", b, (os,os), banks) | ("bl", b, os, bank)
        units = []
        for p in range(NOS):
            units.append(("qp", None, p, [(p % 2) * 4 + i for i in range(4)]))
        if packed:
            units.append(("pf", FB, (0, 1), [0, 1]))
            units.append(("pf", FB, (2, 3), [2, 3]))
        bank_cursor = 4 if packed else 0
        for b in blocks:
            for osl in range(NOS):
                units.append(("bl", b, osl, [bank_cursor % 8]))
                bank_cursor += 1
        last_block = blocks[-1]

        # o_sb buffer per block: quad -> 0-3, packed final -> 5,
        # regular blocks cycle {0,1,2,4} (reuse gated on osc).
        def obuf(b):
            if b < 4:
                return b
            if b == FB:
                return 5
            return [0, 1, 2, 4][blocks.index(b) % 4]

        # ---------------- DMA streams: sync = x, scalar = weights+outs -
        # (a third SWDGE queue on gpsimd was tried for the startup and
        # measured ~7 us SLOWER: Q7 descriptor generation serializes and
        # shares the same SDMA engines.)
        # Both rings share the ~360 GB/s per-core HBM budget for the whole
        # stream (~180 each measured), so: all weight phases are chunked
        # at 2-kb granularity (256 KB) with per-2kb waits — a coarse 1 MB
        # os1 chunk behind os0 on the scalar ring was measured arriving
        # ~2 us after qp1 needed it — and the os3 phase's weights ride
        # the otherwise-lighter sync ring (sync 6.75 MB vs scalar 10.4).
        sy_ring = Ring(nc.sync, "sy")
        sc_ring = Ring(nc.scalar, "sc")
        xq_w = [None] * 8
        w_w = [None] * 32  # index = osl*8 + kb//2
        for c in range(8):  # quad x chunks, 2 kb each (256 KB)
            xsl = slice(c * 1024, (c + 1) * 1024)
            xq_w[c] = sy_ring.dma(xs[:, xsl], xs_d[:, xsl])
        for c in range(8):  # os3 weight chunks on the sync ring
            wsl = slice((3 * KB + 2 * c) * NS, (3 * KB + 2 * (c + 1)) * NS)
            w_w[24 + c] = sy_ring.dma(wt[:, wsl], w_d[:, wsl])
        xb_w = {}
        for i, b in enumerate(border):
            xsl = slice(QX + i * KB * P, QX + (i + 1) * KB * P)
            xb_w[b] = sy_ring.dma(xs[:, xsl], xs_d[:, xsl])
        if packed:
            fbuf = obuf(FB) * D
            for osl in range(NOS):
                rows = slice(0, 64) if osl % 2 == 0 else slice(64, P)
                nc.sync.wait_ge(sems["cf"], 1 + osl // 2)
                sy_ring.dma(
                    out_d[FB * P : FB * P + 64, osl * NS : (osl + 1) * NS],
                    o_sb[rows, fbuf + osl * NS : fbuf + (osl + 1) * NS],
                )
        sy_ring.wait_all()

        for osl in range(3):  # os0-os2 weight chunks on the scalar ring
            for c in range(8):
                wsl = slice(
                    (osl * KB + 2 * c) * NS, (osl * KB + 2 * (c + 1)) * NS
                )
                w_w[osl * 8 + c] = sc_ring.dma(wt[:, wsl], w_d[:, wsl])
        co_thr = 0
        out_w = {}  # block -> wait index of its (last) out DMA
        for b in [0, 1, 2, 3] + blocks:
            buf = obuf(b) * D
            if b == last_block:
                for osl in range(NOS):
                    co_thr += 1
                    nc.scalar.wait_ge(sems["co"], co_thr)
                    out_w[b] = sc_ring.dma(
                        out_d[b * P : (b + 1) * P, osl * NS : (osl + 1) * NS],
                        o_sb[:, buf + osl * NS : buf + (osl + 1) * NS],
                    )
            else:
                co_thr += 1
                nc.scalar.wait_ge(sems["co"], co_thr)
                out_w[b] = sc_ring.dma(
                    out_d[b * P : (b + 1) * P, :], o_sb[:, buf : buf + D]
                )
        sc_ring.wait_all()

        # ---------------- vector engine: PSUM->SBUF copies -------------
        ve = nc.vector
        ve.memset(warm[:], 0.0).then_inc(sems["ws"], 1)
        for u, (kind, b, osl, banks) in enumerate(units):
            ve.wait_ge(sems["mm"], u + 1)
            if kind == "qp":
                p = osl
                last = None
                for ti in range(4):
                    last = ve.tensor_copy(
                        o_sb[:, ti * D + p * NS : ti * D + (p + 1) * NS],
                        ps[banks[ti]][:],
                    )
                last.then_inc(sems["cp"], 1)
                if p == NOS - 1:
                    ve.nop().then_inc(sems["co"], 4)
            elif kind == "pf":
                buf = obuf(b) * D  # dedicated buf 5: no reuse wait needed
                last = None
                for j, o in enumerate(osl):
                    rows = slice(0, 64) if o % 2 == 0 else slice(64, P)
                    last = ve.tensor_copy(
                        o_sb[rows, buf + o * NS : buf + (o + 1) * NS],
                        ps[banks[j]][rows, :],
                    )
                last.then_inc(sems["cp"], 1)
                ve.nop().then_inc(sems["cf"], 1)
            else:  # bl
                bi = blocks.index(b)
                if osl == 0 and bi != 3:
                    # o_sb buf reuse: bufs cycle {0,1,2,4}; bi 0..2 reuse
                    # quad blocks 0..2's bufs, bi 3 gets the fresh buf 4,
                    # bi>=4 reuses regular block bi-4's buf. Wait for that
                    # specific block's out-DMA completion.
                    prev = bi if bi < 3 else blocks[bi - 4]
                    ve.wait_ge(*out_w[prev])
                buf = obuf(b) * D
                last = ve.tensor_copy(
                    o_sb[:, buf + osl * NS : buf + (osl + 1) * NS], ps[banks[0]][:]
                )
                last.then_inc(sems["cp"], 1)
                if b == last_block:
                    ve.nop().then_inc(sems["co"], 1)
                elif osl == 3:
                    ve.nop().then_inc(sems["co"], 1)

        # ---------------- tensor engine: warmups + matmul stream -------
        te = nc.tensor
        te.wait_ge(sems["ws"], 1)
        for _ in range(N_WARM):
            te.matmul(
                ps[0][:64, :64], lhsT=warm[:, :64], rhs=warm[:, :64],
                start=True, stop=True,
            )
        last_use = {}
        for u, (kind, b, osl, banks) in enumerate(units):
            need = 0
            for bk in banks:
                if bk in last_use:
                    need = max(need, last_use[bk] + 1)
                last_use[bk] = u
            if need:
                te.wait_ge(sems["cp"], need)
            if kind == "qp":
                p = osl
                for kb in range(KB):
                    if kb % 2 == 0:
                        te.wait_ge(*w_w[p * 8 + kb // 2])
                        if p == 0:
                            te.wait_ge(*xq_w[kb // 2])
                    for ti in range(4):
                        mm = te.matmul(
                            ps[banks[ti]][:],
                            lhsT=xs[:, xoff(ti, kb) : xoff(ti, kb) + P],
                            rhs=w_sl(wt, p, kb),
                            start=(kb == 0),
                            stop=(kb == KB - 1),
                        )
                mm.then_inc(sems["mm"], 1)
            elif kind == "pf":
                if osl[0] == 0:
                    te.wait_ge(*xb_w[b])
                for kb in range(KB):
                    for j, o in enumerate(osl):
                        dst = ps[banks[j]][:64, :] if o % 2 == 0 else ps[banks[j]][64:, :]
                        mm = te.matmul(
                            dst,
                            lhsT=xs[:, xoff(b, kb) : xoff(b, kb) + 64],
                            rhs=w_sl(wt, o, kb),
                            start=(kb == 0),
                            stop=(kb == KB - 1),
                        )
                mm.then_inc(sems["mm"], 1)
            else:  # bl
                if osl == 0:
                    te.wait_ge(*xb_w[b])
                for kb in range(KB):
                    mm = te.matmul(
                        ps[banks[0]][:],
                        lhsT=xs[:, xoff(b, kb) : xoff(b, kb) + P],
                        rhs=w_sl(wt, osl, kb),
                        start=(kb == 0),
                        stop=(kb == KB - 1),
                    )
                mm.then_inc(sems["mm"], 1)

        # ---------------- teardown: reset sems for the next execution --
        # The NEFF executes more than once per load; sems must return to
        # zero. (A full gpsimd.dma_reset drain costs ~25 us — measured —
        # so only clear the sem values; all DMAs were completion-waited
        # above.)
        # Per-sem clears: a single range sem_clear was measured ~22 us
        # SLOWER (it lowers to a slow ucode path); 21 singles cost ~1 us.
        nc.all_engine_barrier()
        for s in list(sems.values()) + sy_ring.pool + sc_ring.pool:
            nc.gpsimd.sem_clear(s)

        nc.compile()
    return nc


def _get_nc(C, last_m):
    key = (C, last_m)
    if key not in _cache:
        _cache[key] = _build(C, last_m)
    return _cache[key]


def kernel(tokens, weight, exp_ids, _trace=False):
    _ensure_imports()
    from concourse.bass_utils import run_bass_kernel_spmd

    tokens = np.asarray(tokens)
    weight = np.asarray(weight)
    exp_ids = np.asarray(exp_ids)
    T = tokens.shape[0]

    order = np.argsort(exp_ids, kind="stable")
    counts = np.bincount(exp_ids, minlength=E)
    C = max(int(-(-counts.max() // P) * P), 640)
    TB = C // P

    rest = int(counts.max()) - (TB - 1) * P
    last_m = 64 if (TB >= 6 and rest <= 64) else 128
    FB = TB - 1 if last_m == 64 else None
    border = ([FB] if FB is not None else []) + [
        b for b in range(4, TB) if b != FB
    ]

    starts = np.zeros(E + 1, dtype=np.int64)
    np.cumsum(counts, out=starts[1:])

    tokens_c = tokens.astype(np.float16)
    weight_c = weight.astype(np.float16)

    in_maps = []
    for e in range(E):
        idx = order[starts[e] : starts[e + 1]]
        xt = np.zeros((D, C), dtype=np.float16)
        xt[:, : counts[e]] = tokens_c[idx].T
        xt3 = xt.reshape(KB, P, C)
        parts = [np.ascontiguousarray(xt3[:, :, :512].transpose(1, 0, 2)).reshape(P, -1)]
        for b in border:
            parts.append(
                np.ascontiguousarray(
                    xt3[:, :, b * P : (b + 1) * P].transpose(1, 0, 2)
                ).reshape(P, -1)
            )
        xs = np.concatenate(parts, axis=1)
        # weight os-major: [os 4][kb 16][128 p, 512] packed to [128, 4*16*512]
        w4 = weight_c[e].reshape(KB, P, NOS, NS)
        ww = np.ascontiguousarray(w4.transpose(1, 2, 0, 3)).reshape(P, -1)
        in_maps.append({"xs": xs, "w": ww})

    nc = _get_nc(C, last_m)
    res = run_bass_kernel_spmd(
        nc,
        in_maps,
        core_ids=list(range(E)),
        trace=_trace,
        trace_cores=list(range(E)) if _trace else None,
    )

    out = np.empty((T, D), dtype=np.float32)
    for e in range(E):
        idx = order[starts[e] : starts[e + 1]]
        out[idx] = res.results[e]["out"][: counts[e], :].astype(np.float32)
    if _trace:
        return out, res
    return out
